# revision 30
# baseline (speedup 1.0000x reference)
"""MoE feed-forward (8 experts top-2 + 1 shared expert) on 8 Trainium2 cores.

Strategy (expert parallelism, per sharding hint):
  - Host computes the router (8192x1024 @ 1024x8 in f64 — exactly reproduces
    the reference's fp32 top-k selection; verified margins are ~4e-5, far
    above fp32 rounding noise) and dispatches tokens to experts.
  - Core e receives the tokens routed to expert e (gathered, transposed to
    feature-major (D, M), padded to capacity C) plus a 1/8 shard of all
    tokens for the replicated shared expert.
  - Each core runs gate/up/silu/mul/down for its expert segment and its
    shared-expert segment; activations are kept transposed (features on
    partitions, tokens on the free axis) so no on-device transposes needed.
  - Host applies the top-2 combine weights and scatter-adds expert outputs
    plus the shared output back to (B, T, D); aux loss computed on host.

The device kernel does all the heavy FLOPs: ~3*2*(C+1024)*D*H per core.
"""

import os

import numpy as np

# The axon redirect's trace path needs an NTFF hook module that is absent in
# this container; make sure a stray BASS_TRACE env can't send us down it.
os.environ["BASS_NEVER_TRACE"] = "1"

import concourse.bacc as bacc
import concourse.mybir as mybir
from concourse.tile import TileContext
from concourse.bass_utils import run_bass_kernel_spmd

# Problem constants (hardcoded per contract — kernel.py must be self-contained)
B, T, D = 4, 2048, 1024
E, K, S = 8, 2, 1
H = 2048
N = B * T                  # 8192 tokens
NCORES = 8
NS = N // NCORES           # shared-expert tokens per core
P = 128
TF = 512                   # token tile (matmul moving free dim)

# Matmul input dtype. float16 runs at the PE's 1-cycle/row rate (2x fp32r,
# 4x fp32) with ~5e-4 relative error (PSUM accumulation is fp32; all values
# here are O(10) so fp16 range is not a concern). "float32" (exact, 4x
# slower), "float32r" (2.6e-4 err, ~10% slower) are fallbacks.
MM_DTYPE = "float16"

_prog_cache: dict = {}


def _token_tiles(m0, m1):
    """Split [m0, m1) into tiles of at most TF columns."""
    tiles = []
    m = m0
    while m < m1:
        tiles.append((m, min(TF, m1 - m)))
        m += min(TF, m1 - m)
    return tiles


def _build_program(M, C, mm_dtype, repeat=1):
    """Build + compile the per-core Bass program.

    Inputs  : xt (D, M) tokens transposed; ewg/ewu (D, H), ewd (H, D) expert
              weights; swg/swu (D, H), swd (H, D) shared weights.
    Outputs : yt{c} (D, M) fp32 partial outputs, one per H-chunk (summed on
              host).  Expert segment = columns [0, C), shared = [C, M).
    """
    DT = getattr(mybir.dt, mm_dtype)
    f32 = mybir.dt.float32
    n_hc = 1 if mm_dtype in ("bfloat16", "float16") else 2  # H chunking for SBUF fit
    HC = H // n_hc
    HCP = HC // P            # h-blocks per chunk
    DP = D // P              # 8

    nc = bacc.Bacc("TRN2", target_bir_lowering=False, debug=False,
                   num_devices=NCORES)
    xt = nc.dram_tensor("xt", [D, M], DT, kind="ExternalInput")
    ewg = nc.dram_tensor("ewg", [D, H], DT, kind="ExternalInput")
    ewu = nc.dram_tensor("ewu", [D, H], DT, kind="ExternalInput")
    ewd = nc.dram_tensor("ewd", [H, D], DT, kind="ExternalInput")
    swg = nc.dram_tensor("swg", [D, H], DT, kind="ExternalInput")
    swu = nc.dram_tensor("swu", [D, H], DT, kind="ExternalInput")
    swd = nc.dram_tensor("swd", [H, D], DT, kind="ExternalInput")
    yts = [nc.dram_tensor(f"yt{c}", [D, M], f32, kind="ExternalOutput")
           for c in range(n_hc)]

    xt_r = xt.ap().rearrange("(do p) m -> p do m", p=P)
    yt_rs = [y.ap().rearrange("(dp p) m -> p dp m", p=P) for y in yts]

    segs = [(ewg, ewu, ewd, 0, C), (swg, swu, swd, C, M)]
    mult = mybir.AluOpType.mult
    Silu = mybir.ActivationFunctionType.Silu

    with TileContext(nc) as tc:
        with (
            tc.tile_pool(name="w", bufs=1) as wpool,
            tc.tile_pool(name="xp", bufs=2) as xpool,
            tc.tile_pool(name="hp", bufs=2) as hpool,
            tc.tile_pool(name="sp", bufs=3) as spool,
            tc.tile_pool(name="yp", bufs=3) as ypool,
            tc.tile_pool(name="wdp", bufs=2) as wdpool,
            tc.tile_pool(name="ps", bufs=2, space="PSUM") as pspool,
            tc.tile_pool(name="psg", bufs=3, space="PSUM") as gpool,
        ):
          import contextlib
          loop_ctx = tc.For_i(0, repeat, 1) if repeat > 1 else contextlib.nullcontext()
          with loop_ctx:
            def down_proj(h_t, wd_t, mstart, msz, yt_r):
                for db in range(DP):
                    y_ps = pspool.tile([P, TF], f32, tag="y")
                    for hb in range(HCP):
                        nc.tensor.matmul(
                            y_ps[:, :msz],
                            wd_t[:, hb, db * P:(db + 1) * P],
                            h_t[:, hb, :msz],
                            start=(hb == 0), stop=(hb == HCP - 1))
                    y_b = ypool.tile([P, TF], f32, tag="yb")
                    nc.vector.tensor_copy(y_b[:, :msz], y_ps[:, :msz])
                    # per-block store: the last tile's output drains while
                    # its remaining down-proj matmuls still run
                    nc.sync.dma_start(yt_r[:, db, mstart:mstart + msz],
                                      y_b[:, :msz])

            pending = None   # deferred down-proj (software pipeline)
            for (wg_d, wu_d, wd_d, m0, m1) in segs:
                wg_r = wg_d.ap().rearrange("(do p) h -> p do h", p=P)
                wu_r = wu_d.ap().rearrange("(do p) h -> p do h", p=P)
                wd_r = wd_d.ap().rearrange("(ho p) d -> p ho d", p=P)
                for hc in range(n_hc):
                    tiles = _token_tiles(m0, m1)
                    # Prefetch the first x tile ahead of the weight loads so
                    # the first matmuls only wait for x + one wg block.
                    (mstart0, msz0) = tiles[0]
                    x_pre = xpool.tile([P, DP, TF], DT, tag="xt")
                    # Weight loads split per 128-row block, in first-use order;
                    # first x tile interleaved with the first wg half so the
                    # leading matmuls unblock as early as possible.
                    wg_t = wpool.tile([P, DP, HC], DT, tag="wg")
                    wu_t = wpool.tile([P, DP, HC], DT, tag="wu")
                    wd_t = wdpool.tile([P, HCP, D], DT, tag="wd")
                    HH = HC // 2
                    for do in range(DP):
                        nc.sync.dma_start(
                            x_pre[:, do, :msz0],
                            xt_r[:, do, mstart0:mstart0 + msz0])
                        nc.sync.dma_start(
                            wg_t[:, do, :HH], wg_r[:, do, hc * HC:hc * HC + HH])
                    for do in range(DP):
                        nc.sync.dma_start(
                            wu_t[:, do, :HH],
                            wu_r[:, do, hc * HC:hc * HC + HH])
                    for w_t, w_r in ((wg_t, wg_r), (wu_t, wu_r)):
                        for do in range(DP):
                            nc.sync.dma_start(
                                w_t[:, do, HH:], w_r[:, do, hc * HC + HH:
                                                     hc * HC + HC])
                    for ho in range(HCP):
                        nc.sync.dma_start(wd_t[:, ho], wd_r[:, hc * HCP + ho, :])

                    for ti, (mstart, msz) in enumerate(tiles):
                        if ti == 0:
                            x_t = x_pre
                        else:
                            x_t = xpool.tile([P, DP, TF], DT, tag="xt")
                            nc.sync.dma_start(x_t[:, :, :msz],
                                              xt_r[:, :, mstart:mstart + msz])
                        h_t = hpool.tile([P, HCP, TF], DT, tag="ht")
                        for hb in range(HCP):
                            g_ps = gpool.tile([P, TF], f32, tag="g")
                            u_ps = gpool.tile([P, TF], f32, tag="u")
                            for do in range(DP):
                                nc.tensor.matmul(
                                    g_ps[:, :msz],
                                    wg_t[:, do, hb * P:(hb + 1) * P],
                                    x_t[:, do, :msz],
                                    start=(do == 0), stop=(do == DP - 1))
                            for do in range(DP):
                                nc.tensor.matmul(
                                    u_ps[:, :msz],
                                    wu_t[:, do, hb * P:(hb + 1) * P],
                                    x_t[:, do, :msz],
                                    start=(do == 0), stop=(do == DP - 1))
                            s_t = spool.tile([P, TF], f32, tag="sil")
                            nc.scalar.activation(s_t[:, :msz], g_ps[:, :msz], Silu)
                            nc.vector.tensor_tensor(
                                h_t[:, hb, :msz], s_t[:, :msz], u_ps[:, :msz], mult)
                            if hb == 1 and pending is not None:
                                # gate/up of this tile are in flight ahead of
                                # it in PE order; the deferred down-proj now
                                # never stalls PE on the silu/mul chain.
                                down_proj(*pending)
                                pending = None
                        pending = (h_t, wd_t, mstart, msz, yt_rs[hc])
            # wd is double-buffered, so the deferred down-proj of a chunk's
            # last tile safely overlaps the next chunk's weight loads; only
            # the kernel-final tile is flushed here.
            if pending is not None:
                down_proj(*pending)
    nc.compile()
    return nc, n_hc


def _route(flat, Wr):
    """Top-2 routing in f64 (reproduces reference fp32 selection)."""
    logits = flat.astype(np.float64) @ Wr.astype(np.float64)
    n = flat.shape[0]
    ar = np.arange(n)
    i1 = logits.argmax(1)
    v1 = logits[ar, i1]
    l2 = logits.copy()
    l2[ar, i1] = -np.inf
    i2 = l2.argmax(1)
    v2 = l2[ar, i2]
    # softmax over the two selected logits (v1 >= v2)
    b = np.exp(v2 - v1)
    w1 = 1.0 / (1.0 + b)
    w2 = b / (1.0 + b)
    return logits, i1, i2, w1, w2


def kernel(x, Wr, Wg, Wu, Wd, sWg, sWu, sWd):
    flat = np.ascontiguousarray(np.asarray(x, np.float32).reshape(N, D))
    logits, i1, i2, w1, w2 = _route(flat, np.asarray(Wr))

    # per-expert token lists + combine weights
    ids, wts = [], []
    for e in range(E):
        m1 = i1 == e
        m2 = i2 == e
        ids.append(np.concatenate([np.nonzero(m1)[0], np.nonzero(m2)[0]]))
        wts.append(np.concatenate([w1[m1], w2[m2]]).astype(np.float32))
    cap = max(len(ii) for ii in ids)
    C = -(-cap // 8) * 8          # expert capacity (mild alignment only)
    M = C + NS

    key = (M, C, MM_DTYPE)
    if key not in _prog_cache:
        _prog_cache[key] = _build_program(M, C, MM_DTYPE)
    nc, n_hc = _prog_cache[key]

    np_dt = mybir.dt.np(getattr(mybir.dt, MM_DTYPE))
    # shared-expert weights are identical on every core: convert once
    swg_c = np.ascontiguousarray(sWg[0]).astype(np_dt)
    swu_c = np.ascontiguousarray(sWu[0]).astype(np_dt)
    swd_c = np.ascontiguousarray(sWd[0]).astype(np_dt)
    in_maps = []
    for e in range(E):
        xm = np.zeros((M, D), np.float32)
        xm[:len(ids[e])] = flat[ids[e]]
        xm[C:] = flat[e * NS:(e + 1) * NS]
        xt = np.ascontiguousarray(xm.T).astype(np_dt)
        in_maps.append({
            "xt": xt,
            "ewg": np.ascontiguousarray(Wg[e]).astype(np_dt),
            "ewu": np.ascontiguousarray(Wu[e]).astype(np_dt),
            "ewd": np.ascontiguousarray(Wd[e]).astype(np_dt),
            "swg": swg_c,
            "swu": swu_c,
            "swd": swd_c,
        })

    res = run_bass_kernel_spmd(nc, in_maps, list(range(NCORES)))

    out = np.zeros((N, D), np.float32)
    for e in range(E):
        yt = res.results[e]["yt0"]
        for c in range(1, n_hc):
            yt = yt + res.results[e][f"yt{c}"]
        ym = yt.T                               # (M, D)
        ne = len(ids[e])
        out[ids[e]] += wts[e][:, None] * ym[:ne]
        out[e * NS:(e + 1) * NS] += ym[C:]

    # aux loss (host, f64 -> f32)
    lmax = logits.max(1, keepdims=True)
    ex = np.exp(logits - lmax)
    sm = ex / ex.sum(1, keepdims=True)
    Pm = sm.mean(0)
    counts = np.bincount(np.concatenate([i1, i2]), minlength=E).astype(np.float64)
    f = counts / (N * K)
    aux = np.float32((f * Pm).sum() * E)

    return out.reshape(B, T, D), aux


# revision 31
# speedup vs baseline: 1.0000x; 1.0000x over previous
"""MoE feed-forward (8 experts top-2 + 1 shared expert) on 8 Trainium2 cores.

Strategy (expert parallelism, per sharding hint):
  - Host computes the router (8192x1024 @ 1024x8 in f64 — exactly reproduces
    the reference's fp32 top-k selection; verified margins are ~4e-5, far
    above fp32 rounding noise) and dispatches tokens to experts.
  - Core e receives the tokens routed to expert e (gathered, transposed to
    feature-major (D, M), padded to capacity C) plus a 1/8 shard of all
    tokens for the replicated shared expert.
  - Each core runs gate/up/silu/mul/down for its expert segment and its
    shared-expert segment; activations are kept transposed (features on
    partitions, tokens on the free axis) so no on-device transposes needed.
  - Host applies the top-2 combine weights and scatter-adds expert outputs
    plus the shared output back to (B, T, D); aux loss computed on host.

The device kernel does all the heavy FLOPs: ~3*2*(C+1024)*D*H per core.
"""

import os

import numpy as np

# The axon redirect's trace path needs an NTFF hook module that is absent in
# this container; make sure a stray BASS_TRACE env can't send us down it.
os.environ["BASS_NEVER_TRACE"] = "1"

import concourse.bacc as bacc
import concourse.mybir as mybir
from concourse.tile import TileContext
from concourse.bass_utils import run_bass_kernel_spmd

# Problem constants (hardcoded per contract — kernel.py must be self-contained)
B, T, D = 4, 2048, 1024
E, K, S = 8, 2, 1
H = 2048
N = B * T                  # 8192 tokens
NCORES = 8
NS = N // NCORES           # shared-expert tokens per core
P = 128
TF = 512                   # token tile (matmul moving free dim)

# Matmul input dtype. float16 runs at the PE's 1-cycle/row rate (2x fp32r,
# 4x fp32) with ~5e-4 relative error (PSUM accumulation is fp32; all values
# here are O(10) so fp16 range is not a concern). "float32" (exact, 4x
# slower), "float32r" (2.6e-4 err, ~10% slower) are fallbacks.
MM_DTYPE = "float16"

_prog_cache: dict = {}


def _token_tiles(m0, m1):
    """Split [m0, m1) into tiles of at most TF columns."""
    tiles = []
    m = m0
    while m < m1:
        tiles.append((m, min(TF, m1 - m)))
        m += min(TF, m1 - m)
    return tiles


def _build_program(M, C, mm_dtype, repeat=1):
    """Build + compile the per-core Bass program.

    Inputs  : xt (D, M) tokens transposed; ewg/ewu (D, H), ewd (H, D) expert
              weights; swg/swu (D, H), swd (H, D) shared weights.
    Outputs : yt{c} (D, M) fp32 partial outputs, one per H-chunk (summed on
              host).  Expert segment = columns [0, C), shared = [C, M).
    """
    DT = getattr(mybir.dt, mm_dtype)
    f32 = mybir.dt.float32
    n_hc = 1 if mm_dtype in ("bfloat16", "float16") else 2  # H chunking for SBUF fit
    HC = H // n_hc
    HCP = HC // P            # h-blocks per chunk
    DP = D // P              # 8

    nc = bacc.Bacc("TRN2", target_bir_lowering=False, debug=False,
                   num_devices=NCORES)
    xt = nc.dram_tensor("xt", [D, M], DT, kind="ExternalInput")
    ewg = nc.dram_tensor("ewg", [D, H], DT, kind="ExternalInput")
    ewu = nc.dram_tensor("ewu", [D, H], DT, kind="ExternalInput")
    ewd = nc.dram_tensor("ewd", [H, D], DT, kind="ExternalInput")
    swg = nc.dram_tensor("swg", [D, H], DT, kind="ExternalInput")
    swu = nc.dram_tensor("swu", [D, H], DT, kind="ExternalInput")
    swd = nc.dram_tensor("swd", [H, D], DT, kind="ExternalInput")
    yts = [nc.dram_tensor(f"yt{c}", [D, M], f32, kind="ExternalOutput")
           for c in range(n_hc)]

    xt_r = xt.ap().rearrange("(do p) m -> p do m", p=P)
    yt_rs = [y.ap().rearrange("(dp p) m -> p dp m", p=P) for y in yts]

    segs = [(ewg, ewu, ewd, 0, C), (swg, swu, swd, C, M)]
    mult = mybir.AluOpType.mult
    Silu = mybir.ActivationFunctionType.Silu

    with TileContext(nc) as tc:
        with (
            tc.tile_pool(name="w", bufs=1) as wpool,
            tc.tile_pool(name="xp", bufs=2) as xpool,
            tc.tile_pool(name="hp", bufs=2) as hpool,
            tc.tile_pool(name="sp", bufs=3) as spool,
            tc.tile_pool(name="yp", bufs=3) as ypool,
            tc.tile_pool(name="wdp", bufs=2) as wdpool,
            tc.tile_pool(name="ps", bufs=2, space="PSUM") as pspool,
            tc.tile_pool(name="psg", bufs=3, space="PSUM") as gpool,
        ):
          import contextlib
          loop_ctx = tc.For_i(0, repeat, 1) if repeat > 1 else contextlib.nullcontext()
          with loop_ctx:
            def down_proj(h_t, wd_t, mstart, msz, yt_r):
                for db in range(DP):
                    y_ps = pspool.tile([P, TF], f32, tag="y")
                    for hb in range(HCP):
                        nc.tensor.matmul(
                            y_ps[:, :msz],
                            wd_t[:, hb, db * P:(db + 1) * P],
                            h_t[:, hb, :msz],
                            start=(hb == 0), stop=(hb == HCP - 1))
                    y_b = ypool.tile([P, TF], f32, tag="yb")
                    nc.vector.tensor_copy(y_b[:, :msz], y_ps[:, :msz])
                    # per-block store: the last tile's output drains while
                    # its remaining down-proj matmuls still run
                    nc.sync.dma_start(yt_r[:, db, mstart:mstart + msz],
                                      y_b[:, :msz])

            pending = None   # deferred down-proj (software pipeline)
            for (wg_d, wu_d, wd_d, m0, m1) in segs:
                wg_r = wg_d.ap().rearrange("(do p) h -> p do h", p=P)
                wu_r = wu_d.ap().rearrange("(do p) h -> p do h", p=P)
                wd_r = wd_d.ap().rearrange("(ho p) d -> p ho d", p=P)
                for hc in range(n_hc):
                    tiles = _token_tiles(m0, m1)
                    # Prefetch the first x tile ahead of the weight loads so
                    # the first matmuls only wait for x + one wg block.
                    (mstart0, msz0) = tiles[0]
                    x_pre = xpool.tile([P, DP, TF], DT, tag="xt")
                    # Weight loads split per 128-row block, in first-use order;
                    # first x tile interleaved with the first wg half so the
                    # leading matmuls unblock as early as possible.
                    wg_t = wpool.tile([P, DP, HC], DT, tag="wg")
                    wu_t = wpool.tile([P, DP, HC], DT, tag="wu")
                    wd_t = wdpool.tile([P, HCP, D], DT, tag="wd")
                    HH = HC // 2
                    for do in range(DP):
                        nc.sync.dma_start(
                            x_pre[:, do, :msz0],
                            xt_r[:, do, mstart0:mstart0 + msz0])
                        nc.sync.dma_start(
                            wg_t[:, do, :HH], wg_r[:, do, hc * HC:hc * HC + HH])
                    for do in range(DP):
                        nc.sync.dma_start(
                            wu_t[:, do, :HH],
                            wu_r[:, do, hc * HC:hc * HC + HH])
                    for w_t, w_r in ((wg_t, wg_r), (wu_t, wu_r)):
                        for do in range(DP):
                            nc.sync.dma_start(
                                w_t[:, do, HH:], w_r[:, do, hc * HC + HH:
                                                     hc * HC + HC])
                    for ho in range(HCP):
                        nc.sync.dma_start(wd_t[:, ho], wd_r[:, hc * HCP + ho, :])

                    for ti, (mstart, msz) in enumerate(tiles):
                        if ti == 0:
                            x_t = x_pre
                        else:
                            x_t = xpool.tile([P, DP, TF], DT, tag="xt")
                            nc.sync.dma_start(x_t[:, :, :msz],
                                              xt_r[:, :, mstart:mstart + msz])
                        h_t = hpool.tile([P, HCP, TF], DT, tag="ht")
                        if ti == 0:
                            # First tile of a chunk: run gates SKEW h-blocks
                            # ahead of ups so the in-order PE is insulated
                            # from the wu stream still arriving.
                            if pending is not None:
                                down_proj(*pending)
                                pending = None
                            SKEW = 2
                            s_ts = {}
                            for hb in range(HCP + SKEW):
                                if hb < HCP:
                                    g_ps = gpool.tile([P, TF], f32, tag="g")
                                    for do in range(DP):
                                        nc.tensor.matmul(
                                            g_ps[:, :msz],
                                            wg_t[:, do, hb * P:(hb + 1) * P],
                                            x_t[:, do, :msz],
                                            start=(do == 0), stop=(do == DP - 1))
                                    s_t = spool.tile([P, TF], f32, tag="sil")
                                    nc.scalar.activation(
                                        s_t[:, :msz], g_ps[:, :msz], Silu)
                                    s_ts[hb] = s_t
                                k = hb - SKEW
                                if k >= 0:
                                    u_ps = gpool.tile([P, TF], f32, tag="u")
                                    for do in range(DP):
                                        nc.tensor.matmul(
                                            u_ps[:, :msz],
                                            wu_t[:, do, k * P:(k + 1) * P],
                                            x_t[:, do, :msz],
                                            start=(do == 0), stop=(do == DP - 1))
                                    nc.vector.tensor_tensor(
                                        h_t[:, k, :msz], s_ts.pop(k)[:, :msz],
                                        u_ps[:, :msz], mult)
                        else:
                            for hb in range(HCP):
                                g_ps = gpool.tile([P, TF], f32, tag="g")
                                u_ps = gpool.tile([P, TF], f32, tag="u")
                                for do in range(DP):
                                    nc.tensor.matmul(
                                        g_ps[:, :msz],
                                        wg_t[:, do, hb * P:(hb + 1) * P],
                                        x_t[:, do, :msz],
                                        start=(do == 0), stop=(do == DP - 1))
                                for do in range(DP):
                                    nc.tensor.matmul(
                                        u_ps[:, :msz],
                                        wu_t[:, do, hb * P:(hb + 1) * P],
                                        x_t[:, do, :msz],
                                        start=(do == 0), stop=(do == DP - 1))
                                s_t = spool.tile([P, TF], f32, tag="sil")
                                nc.scalar.activation(s_t[:, :msz], g_ps[:, :msz], Silu)
                                nc.vector.tensor_tensor(
                                    h_t[:, hb, :msz], s_t[:, :msz], u_ps[:, :msz], mult)
                                if hb == 1 and pending is not None:
                                    down_proj(*pending)
                                    pending = None
                        pending = (h_t, wd_t, mstart, msz, yt_rs[hc])
            # wd is double-buffered, so the deferred down-proj of a chunk's
            # last tile safely overlaps the next chunk's weight loads; only
            # the kernel-final tile is flushed here.
            if pending is not None:
                down_proj(*pending)
    nc.compile()
    return nc, n_hc


def _route(flat, Wr):
    """Top-2 routing in f64 (reproduces reference fp32 selection)."""
    logits = flat.astype(np.float64) @ Wr.astype(np.float64)
    n = flat.shape[0]
    ar = np.arange(n)
    i1 = logits.argmax(1)
    v1 = logits[ar, i1]
    l2 = logits.copy()
    l2[ar, i1] = -np.inf
    i2 = l2.argmax(1)
    v2 = l2[ar, i2]
    # softmax over the two selected logits (v1 >= v2)
    b = np.exp(v2 - v1)
    w1 = 1.0 / (1.0 + b)
    w2 = b / (1.0 + b)
    return logits, i1, i2, w1, w2


def kernel(x, Wr, Wg, Wu, Wd, sWg, sWu, sWd):
    flat = np.ascontiguousarray(np.asarray(x, np.float32).reshape(N, D))
    logits, i1, i2, w1, w2 = _route(flat, np.asarray(Wr))

    # per-expert token lists + combine weights
    ids, wts = [], []
    for e in range(E):
        m1 = i1 == e
        m2 = i2 == e
        ids.append(np.concatenate([np.nonzero(m1)[0], np.nonzero(m2)[0]]))
        wts.append(np.concatenate([w1[m1], w2[m2]]).astype(np.float32))
    cap = max(len(ii) for ii in ids)
    C = -(-cap // 8) * 8          # expert capacity (mild alignment only)
    M = C + NS

    key = (M, C, MM_DTYPE)
    if key not in _prog_cache:
        _prog_cache[key] = _build_program(M, C, MM_DTYPE)
    nc, n_hc = _prog_cache[key]

    np_dt = mybir.dt.np(getattr(mybir.dt, MM_DTYPE))
    # shared-expert weights are identical on every core: convert once
    swg_c = np.ascontiguousarray(sWg[0]).astype(np_dt)
    swu_c = np.ascontiguousarray(sWu[0]).astype(np_dt)
    swd_c = np.ascontiguousarray(sWd[0]).astype(np_dt)
    in_maps = []
    for e in range(E):
        xm = np.zeros((M, D), np.float32)
        xm[:len(ids[e])] = flat[ids[e]]
        xm[C:] = flat[e * NS:(e + 1) * NS]
        xt = np.ascontiguousarray(xm.T).astype(np_dt)
        in_maps.append({
            "xt": xt,
            "ewg": np.ascontiguousarray(Wg[e]).astype(np_dt),
            "ewu": np.ascontiguousarray(Wu[e]).astype(np_dt),
            "ewd": np.ascontiguousarray(Wd[e]).astype(np_dt),
            "swg": swg_c,
            "swu": swu_c,
            "swd": swd_c,
        })

    res = run_bass_kernel_spmd(nc, in_maps, list(range(NCORES)))

    out = np.zeros((N, D), np.float32)
    for e in range(E):
        yt = res.results[e]["yt0"]
        for c in range(1, n_hc):
            yt = yt + res.results[e][f"yt{c}"]
        ym = yt.T                               # (M, D)
        ne = len(ids[e])
        out[ids[e]] += wts[e][:, None] * ym[:ne]
        out[e * NS:(e + 1) * NS] += ym[C:]

    # aux loss (host, f64 -> f32)
    lmax = logits.max(1, keepdims=True)
    ex = np.exp(logits - lmax)
    sm = ex / ex.sum(1, keepdims=True)
    Pm = sm.mean(0)
    counts = np.bincount(np.concatenate([i1, i2]), minlength=E).astype(np.float64)
    f = counts / (N * K)
    aux = np.float32((f * Pm).sum() * E)

    return out.reshape(B, T, D), aux
